# revision 74
# baseline (speedup 1.0000x reference)
"""AlphaPermutationLayer Trainium2 kernel (v2).

out[i, j] = sum_k softmax(alpha/T)[k] * (perm[k, i] == j),  N=2048, K=64.

Sharding: output ROWS across the 8 cores (row i depends only on perm[:, i]
and alpha — no collective).  Per core (256 rows), digit-split
j = jq*64 + jf (jq in [0,32), jf in [0,64)); one matmul per row i:
    out_i[jq, jf] = sum_k A_i[k, jq] * B_i[k, jf]
with A = alpha-scaled onehot(perm>>6) stationary ([64, 32], LDW 32 cols)
and B = onehot(perm&63) moving ([64, 64]).  Rows are processed two-per-
partition-set: partition p = k + 64*h holds row half h, so DVE one-hot
builds use all 128 lanes while each matmul contracts 64 partitions at
tile_position (64h, 32g) — 4 col-groups give concurrent matmuls.  Single
bf16 pass (no hi/lo): alpha rounding ~2e-3 rel, gate is 2e-2.  PSUM holds
the whole 2MB per-core output; ACT evacuates with the fused 1/S softmax
normalization; strided DMAs (256B runs) write DRAM.
"""

import os
import sys

sys.path.insert(0, "/opt/trn_rl_repo")

import numpy as np

N = 2048
K = 64
NCORES = 8
ROWS = N // NCORES          # 256 rows per core
Q = 32                      # stationary digit width (jq)
F = 64                      # moving digit width (jf)
CW = 32                     # i2 chunk width (4 chunks of 32)
IL = 4                      # low i2 bits kept innermost in one-hot layout

LAST_EXEC_NS = None
LAST_RESULTS = None

_cached = {}


def _build_bass():
    import concourse.tile as tile
    from concourse import bacc, mybir

    fp32 = mybir.dt.float32
    bf16 = mybir.dt.bfloat16
    i16 = mybir.dt.int16
    Copy = mybir.ActivationFunctionType.Copy
    Exp = mybir.ActivationFunctionType.Exp
    IsEq = mybir.AluOpType.is_equal

    nc = bacc.Bacc()

    ph_ext = nc.declare_dram_parameter("ph", [128, 128], i16, isOutput=False)
    pl_ext = nc.declare_dram_parameter("pl", [128, 128], i16, isOutput=False)
    at_ext = nc.declare_dram_parameter("altp", [128, 2], fp32, isOutput=False)
    out_ext = nc.declare_dram_parameter("out", [ROWS, N], fp32, isOutput=True)

    with tile.TileContext(nc) as tc:
        with (
            tc.tile_pool(name="sbuf", bufs=1) as sb,
            tc.tile_pool(name="stage", bufs=10) as stp,
            tc.tile_pool(name="smax_psum", bufs=1, space="PSUM") as psmax,
            tc.tile_pool(name="psum", bufs=7, space="PSUM") as pp,
        ):
            # ---- input loads: 2 DMAs; iotas generated on-chip ---------------
            # One-hot tensors are laid out [p, i2h, digit, i2l] with the LOW
            # i2 bits innermost (IL=4): the is_equal in0 is just ph/pl viewed
            # [p, i2h, 1->digit, i2l] — real data, stride-1 inner, 2x DVE
            # mode with NO host expansion.  The matmul then reads digit cols
            # at stride IL*2 = 8B (2 per 16B SBUF line), cutting the line
            # thrash that throttles concurrent DVE/ACT ops.
            ph_t = sb.tile([128, 128], i16)
            pl_t = sb.tile([128, 128], i16)
            at_t = sb.tile([128, 2], fp32)
            nc.sync.dma_start(out=ph_t[:, 0:64], in_=ph_ext[:, 0:64])
            nc.scalar.dma_start(out=ph_t[:, 64:128], in_=ph_ext[:, 64:128])
            nc.sync.dma_start(out=pl_t[:], in_=pl_ext[:])
            nc.scalar.dma_start(out=at_t[:], in_=at_ext[:])
            ph_v = ph_t[:].rearrange("p (ih il) -> p ih il", il=IL)
            pl_v = pl_t[:].rearrange("p (ih il) -> p ih il", il=IL)
            iq_t = sb.tile([128, Q, IL], i16)   # [p, q, il] = q
            if_t = sb.tile([128, F, IL], i16)   # [p, f, il] = f
            nc.gpsimd.iota(iq_t[:], pattern=[[1, Q], [0, IL]], channel_multiplier=0)
            nc.gpsimd.iota(if_t[:], pattern=[[1, F], [0, IL]], channel_multiplier=0)
            al_t = at_t[:, 0:1]
            tp_t = at_t[:, 1:2]

            # ---- softmax head ----------------------------------------------
            # e = exp(alpha/T) unnormalized; S recovered via matmul with 0.5
            # (partitions hold k twice), 1/S applied at evacuation.
            rt_t = sb.tile([128, 1], fp32)
            e_t = sb.tile([128, 1], fp32)
            ln2_t = sb.tile([128, 1], fp32)
            prime_t = sb.tile([128, 1], fp32)
            half_col = sb.tile([128, 1], fp32)
            ones_row = sb.tile([1, 128], fp32)
            r_t = sb.tile([1, 1], fp32)
            rs_t = sb.tile([128, 1], fp32)
            scr_t = sb.tile([128, 512], bf16)   # warmup scratch (uninit junk)
            nc.vector.memset(scr_t[:, 0:2], 1.0)
            nc.vector.memset(ln2_t[:], float(np.log(2.0)))
            nc.vector.memset(half_col[:], 0.5)
            nc.vector.memset(ones_row[:], 1.0)
            # dep-free ACT op: hoists the one-time activation-table load off
            # the exp critical path.
            nc.scalar.activation(out=prime_t[:], in_=ln2_t[:], func=Exp)
            warm_ps = psmax.tile([1, 512], fp32, tag="smax")
            sum_ps = psmax.tile([1, 1], fp32, tag="smax")
            # HAM pre-warm: a few WIDE dep-free matmuls (~3us of PE busy in 7
            # instructions) so the clock gate reaches 8/8 before the real
            # stream without clogging the PE queue ahead of the softmax sum.
            for _ in range(7):
                nc.tensor.matmul(
                    warm_ps[:], lhsT=scr_t[:, 0:1], rhs=scr_t[:],
                    start=True, stop=True,
                )

            IH = 128 // IL                       # i2h extent (32)
            CH = IH // 4                         # i2h per chunk (8)
            a_t = sb.tile([128, IH, Q, IL], bf16)  # [p, i2h, jq, i2l]
            a_s = sb.tile([128, IH, Q, IL], bf16)  # alpha-scaled
            b_t = sb.tile([128, IH, F, IL], bf16)  # [p, i2h, jf, i2l]

            # DRAM view: row i = 32b + 4s + g, col j = q*64 + f;
            # psum partition = 32g + q, psum free = 64s + f.  With g the LOW
            # row bits, the DRAM dims (g, q) merge to one stride-64 dim, so
            # the whole bank drains in ONE 3-dim dma_start.
            oview = out_ext[:].rearrange(
                "(b s g) (q f) -> b g q s f", b=8, s=8, g=4, q=Q, f=F
            )

            banks = [None] * 8
            rb_ps = []

            def emit_chunk(c):
                ih = slice(CH * c, CH * c + CH)
                nc.vector.tensor_tensor(
                    out=a_t[:, ih],
                    in0=ph_v[:, ih].unsqueeze(2).to_broadcast([128, CH, Q, IL]),
                    in1=iq_t[:].unsqueeze(1).to_broadcast([128, CH, Q, IL]),
                    op=IsEq,
                )
                if c == 0:
                    # alpha chain between the two builds: altp has landed by
                    # the time A0 retires, and e_t is ready before a_s0.
                    nc.vector.reciprocal(out=rt_t[:], in_=tp_t[:])
                    nc.scalar.activation(
                        out=e_t[:], in_=al_t[:], func=Exp, scale=rt_t[:]
                    )
                    nc.tensor.matmul(
                        sum_ps[:], lhsT=e_t[:], rhs=half_col[:],
                        start=True, stop=True,
                    )
                nc.vector.tensor_tensor(
                    out=b_t[:, ih],
                    in0=pl_v[:, ih].unsqueeze(2).to_broadcast([128, CH, F, IL]),
                    in1=if_t[:].unsqueeze(1).to_broadcast([128, CH, F, IL]),
                    op=IsEq,
                )
                as_eng = (
                    nc.gpsimd
                    if os.environ.get("KERNEL_AS_GP", "0") == "1"
                    else nc.vector
                )
                as_eng.tensor_scalar(
                    out=a_s[:, ih], in0=a_t[:, ih], scalar1=e_t[:],
                    scalar2=None, op0=mybir.AluOpType.mult,
                )
                if c == 0:
                    # softmax tail fully BEFORE the row matmuls so rs_t is
                    # ready when bank 0 drains (sum_ps done during builds).
                    nc.vector.reciprocal(out=r_t[:], in_=sum_ps[:])
                    rb = psmax.tile([128, 1], fp32, tag="smax", name="rb_ps")
                    rb_ps.append(rb)
                    nc.tensor.matmul(
                        rb[:], lhsT=ones_row[:], rhs=r_t[:],
                        start=True, stop=True,
                    )
                    nc.vector.tensor_copy(out=rs_t[:], in_=rb[:])
                for h in range(2):
                    banks[c + 4 * h] = pp.tile(
                        [128, 8, F], fp32, tag="bank", name=f"bank{c}_{h}"
                    )
                # h OUTER: consecutive matmuls stay in one row-group, so the
                # PE runs a single serialized stream (g-alternation only
                # overlaps LDW).  Concurrent h-interleaved streams are faster
                # on paper but saturate SBUF read bw and throttle DVE/ACT
                # ops 2-3x (measured), a net loss.
                for h in range(2):
                    for s in range(8):
                        for g in range(4):
                            # row r = 128h + i2, i2 = 4*(CH*c+s) + g
                            kp = slice(64 * h, 64 * h + 64)
                            nc.tensor.matmul(
                                banks[c + 4 * h][32 * g : 32 * g + 32, s],
                                lhsT=a_s[kp, CH * c + s, :, g],
                                rhs=b_t[kp, CH * c + s, :, g],
                                start=True,
                                stop=True,
                                tile_position=(64 * h, 32 * g),
                            )
            def emit_drain(c):
                # Chunk 0's banks drain in s-halves so the output DMA ramps
                # ~1.5us earlier; later banks go whole (fewer issues).
                for h in range(2):
                    bi = c + 4 * h
                    # all drain DMAs issue from the sync ring: its sequencer
                    # is idle during the drain, while the scalar sequencer
                    # runs the evac COPYs back-to-back.
                    eng = nc.sync
                    parts = ((slice(0, 4), slice(4, 8)) if c == 0
                             else (slice(0, 8),))
                    for sp in parts:
                        nsl = sp.stop - sp.start
                        stage = stp.tile(
                            [128, nsl, F], fp32, tag="stage", name="stage"
                        )
                        nc.scalar.activation(
                            out=stage[:], in_=banks[bi][:, sp], func=Copy,
                            scale=rs_t[:],
                        )
                        eng.dma_start(out=oview[bi][:, :, sp], in_=stage[:])

            for c in range(4):
                emit_chunk(c)
                if c >= 1:
                    emit_drain(c - 1)
            emit_drain(3)

    if not nc.is_finalized():
        nc.finalize()
    return nc


def _prep_inputs(alpha_weights, perm_vectors, temperature):
    a = np.asarray(alpha_weights, dtype=np.float32).reshape(K)
    T = np.asarray(temperature, dtype=np.float32).reshape(())
    perm = np.asarray(perm_vectors).astype(np.int64).reshape(K, N)
    ph = (perm >> 6).astype(np.int16)
    pl = (perm & 63).astype(np.int16)
    al_t = np.concatenate([a, a])[:, None].copy()          # [128, 1]
    tp_t = np.full((128, 1), T, dtype=np.float32)
    in_maps = []
    for c in range(NCORES):
        # partition p = k + 64*h, column i2: row r = 128*h + i2 of this core
        phc = ph[:, c * ROWS : (c + 1) * ROWS].reshape(K, 2, 128)
        plc = pl[:, c * ROWS : (c + 1) * ROWS].reshape(K, 2, 128)
        in_maps.append(
            {
                "ph": phc.transpose(1, 0, 2).reshape(128, 128).copy(),
                "pl": plc.transpose(1, 0, 2).reshape(128, 128).copy(),
                "altp": np.concatenate([al_t, tp_t], axis=1).copy(),
            }
        )
    return in_maps


def _install_ntff_hook():
    """Provide antenv.axon_hooks (missing in this image) so that
    run_bass_kernel_spmd(trace=True) can capture NTFF profiles via the
    axon PJRT .so (same mechanism as trn_agent_boot.trn_boot)."""
    import contextlib
    import ctypes
    import types

    try:
        from antenv.axon_hooks import get_axon_ntff_profile_hook  # noqa: F401

        return True
    except ImportError:
        pass
    so_path = "/opt/axon/libaxon_pjrt.so"
    if not os.path.exists(so_path):
        return False
    lib = ctypes.CDLL(so_path)
    if not hasattr(lib, "axon_start_nrt_profile"):
        return False
    lib.axon_start_nrt_profile.argtypes = [
        ctypes.POINTER(ctypes.c_int64),
        ctypes.c_size_t,
    ]
    lib.axon_start_nrt_profile.restype = ctypes.c_int64
    lib.axon_stop_nrt_profile.argtypes = [ctypes.c_char_p]
    lib.axon_stop_nrt_profile.restype = ctypes.c_int64

    @contextlib.contextmanager
    def _hook(output_dir, device_ids):
        import jax

        jax.devices()
        if device_ids:
            ids = (ctypes.c_int64 * len(device_ids))(*device_ids)
            rc = lib.axon_start_nrt_profile(ids, len(device_ids))
        else:
            rc = lib.axon_start_nrt_profile(None, 0)
        if rc != 0:
            raise RuntimeError(f"axon_start_nrt_profile rc={rc}")
        try:
            yield
        finally:
            n = lib.axon_stop_nrt_profile(str(output_dir).encode())
            print(f"ntff profile: {n} file(s) written to {output_dir}")

    import antenv

    mod = types.ModuleType("antenv.axon_hooks")
    mod.get_axon_ntff_profile_hook = lambda: _hook
    mod.set_axon_ntff_profile_hook = lambda h: None
    sys.modules["antenv.axon_hooks"] = mod
    antenv.axon_hooks = mod
    return True


def kernel(alpha_weights, perm_vectors, temperature):
    global LAST_EXEC_NS, LAST_RESULTS
    from concourse.bass_utils import run_bass_kernel_spmd

    if "nc" not in _cached:
        _cached["nc"] = _build_bass()
    nc = _cached["nc"]
    in_maps = _prep_inputs(alpha_weights, perm_vectors, temperature)
    core_ids = list(range(NCORES))
    trace = os.environ.get("KERNEL_TRACE", "0") == "1"
    if trace:
        trace = _install_ntff_hook()
    try:
        res = run_bass_kernel_spmd(nc, in_maps, core_ids, trace=trace)
    except Exception:
        if not trace:
            raise
        res = run_bass_kernel_spmd(nc, in_maps, core_ids, trace=False)
    LAST_EXEC_NS = res.exec_time_ns
    LAST_RESULTS = res
    out = np.concatenate([res.results[c]["out"] for c in range(NCORES)], axis=0)
    return out.astype(np.float32)


if __name__ == "__main__":
    rng = np.random.default_rng(0)
    a = rng.standard_normal(K).astype(np.float32)
    perm = np.stack([rng.permutation(N) for _ in range(K)]).astype(np.int64)
    T = np.ones((), np.float32)
    out = kernel(a, perm, T)
    # numpy reference
    al = np.exp(a / T - (a / T).max())
    al /= al.sum()
    exp = np.zeros((N, N), np.float32)
    np.add.at(exp, (np.broadcast_to(np.arange(N), (K, N)), perm), al[:, None])
    print("max abs err:", np.abs(out - exp).max(), "max ref:", np.abs(exp).max())
    print("exec ns:", LAST_EXEC_NS)


# revision 75
# speedup vs baseline: 2.7313x; 2.7313x over previous
"""AlphaPermutationLayer Trainium2 kernel (v2).

out[i, j] = sum_k softmax(alpha/T)[k] * (perm[k, i] == j),  N=2048, K=64.

Sharding: output ROWS across the 8 cores (row i depends only on perm[:, i]
and alpha — no collective).  Per core (256 rows), digit-split
j = jq*64 + jf (jq in [0,32), jf in [0,64)); one matmul per row i:
    out_i[jq, jf] = sum_k A_i[k, jq] * B_i[k, jf]
with A = alpha-scaled onehot(perm>>6) stationary ([64, 32], LDW 32 cols)
and B = onehot(perm&63) moving ([64, 64]).  Rows are processed two-per-
partition-set: partition p = k + 64*h holds row half h, so DVE one-hot
builds use all 128 lanes while each matmul contracts 64 partitions at
tile_position (64h, 32g) — 4 col-groups give concurrent matmuls.  Single
bf16 pass (no hi/lo): alpha rounding ~2e-3 rel, gate is 2e-2.  PSUM holds
the whole 2MB per-core output; ACT evacuates with the fused 1/S softmax
normalization; strided DMAs (256B runs) write DRAM.
"""

import os
import sys

sys.path.insert(0, "/opt/trn_rl_repo")

import numpy as np

N = 2048
K = 64
NCORES = 8
ROWS = N // NCORES          # 256 rows per core
Q = 32                      # stationary digit width (jq)
F = 64                      # moving digit width (jf)
CW = 32                     # i2 chunk width (4 chunks of 32)
IL = 4                      # low i2 bits kept innermost in one-hot layout

LAST_EXEC_NS = None
LAST_RESULTS = None

_cached = {}


def _build_bass():
    import concourse.tile as tile
    from concourse import bacc, mybir

    fp32 = mybir.dt.float32
    bf16 = mybir.dt.bfloat16
    i16 = mybir.dt.int16
    Copy = mybir.ActivationFunctionType.Copy
    Exp = mybir.ActivationFunctionType.Exp
    IsEq = mybir.AluOpType.is_equal

    nc = bacc.Bacc()

    ph_ext = nc.declare_dram_parameter("ph", [128, 128], i16, isOutput=False)
    pl_ext = nc.declare_dram_parameter("pl", [128, 128], i16, isOutput=False)
    at_ext = nc.declare_dram_parameter("altp", [128, 2], fp32, isOutput=False)
    out_ext = nc.declare_dram_parameter("out", [ROWS, N], fp32, isOutput=True)

    with tile.TileContext(nc) as tc:
        with (
            tc.tile_pool(name="sbuf", bufs=1) as sb,
            tc.tile_pool(name="stage", bufs=10) as stp,
            tc.tile_pool(name="smax_psum", bufs=1, space="PSUM") as psmax,
            tc.tile_pool(name="psum", bufs=7, space="PSUM") as pp,
        ):
            # ---- input loads: 2 DMAs; iotas generated on-chip ---------------
            # One-hot tensors are laid out [p, i2h, digit, i2l] with the LOW
            # i2 bits innermost (IL=4): the is_equal in0 is just ph/pl viewed
            # [p, i2h, 1->digit, i2l] — real data, stride-1 inner, 2x DVE
            # mode with NO host expansion.  The matmul then reads digit cols
            # at stride IL*2 = 8B (2 per 16B SBUF line), cutting the line
            # thrash that throttles concurrent DVE/ACT ops.
            ph_t = sb.tile([128, 128], i16)
            pl_t = sb.tile([128, 128], i16)
            at_t = sb.tile([128, 2], fp32)
            nc.sync.dma_start(out=ph_t[:, 0:64], in_=ph_ext[:, 0:64])
            nc.scalar.dma_start(out=ph_t[:, 64:128], in_=ph_ext[:, 64:128])
            nc.gpsimd.dma_start(out=pl_t[:], in_=pl_ext[:])
            nc.scalar.dma_start(out=at_t[:], in_=at_ext[:])
            ph_v = ph_t[:].rearrange("p (ih il) -> p ih il", il=IL)
            pl_v = pl_t[:].rearrange("p (ih il) -> p ih il", il=IL)
            iq_t = sb.tile([128, Q, IL], i16)   # [p, q, il] = q
            if_t = sb.tile([128, F, IL], i16)   # [p, f, il] = f
            nc.gpsimd.iota(iq_t[:], pattern=[[1, Q], [0, IL]], channel_multiplier=0)
            nc.gpsimd.iota(if_t[:], pattern=[[1, F], [0, IL]], channel_multiplier=0)
            al_t = at_t[:, 0:1]
            tp_t = at_t[:, 1:2]

            # ---- softmax head ----------------------------------------------
            # e = exp(alpha/T) unnormalized; S recovered via matmul with 0.5
            # (partitions hold k twice), 1/S applied at evacuation.
            rt_t = sb.tile([128, 1], fp32)
            e_t = sb.tile([128, 1], fp32)
            ln2_t = sb.tile([128, 1], fp32)
            prime_t = sb.tile([128, 1], fp32)
            half_col = sb.tile([128, 1], fp32)
            ones_row = sb.tile([1, 128], fp32)
            r_t = sb.tile([1, 1], fp32)
            rs_t = sb.tile([128, 1], fp32)
            scr_t = sb.tile([128, 512], bf16)   # warmup scratch (uninit junk)
            nc.vector.memset(scr_t[:, 0:2], 1.0)
            nc.vector.memset(ln2_t[:], float(np.log(2.0)))
            nc.vector.memset(half_col[:], 0.5)
            nc.vector.memset(ones_row[:], 1.0)
            # dep-free ACT op: hoists the one-time activation-table load off
            # the exp critical path.
            nc.scalar.activation(out=prime_t[:], in_=ln2_t[:], func=Exp)
            warm_ps = psmax.tile([1, 512], fp32, tag="smax")
            sum_ps = psmax.tile([1, 1], fp32, tag="smax")
            # HAM pre-warm: a few WIDE dep-free matmuls (~3us of PE busy in 7
            # instructions) so the clock gate reaches 8/8 before the real
            # stream without clogging the PE queue ahead of the softmax sum.
            for _ in range(7):
                nc.tensor.matmul(
                    warm_ps[:], lhsT=scr_t[:, 0:1], rhs=scr_t[:],
                    start=True, stop=True,
                )

            IH = 128 // IL                       # i2h extent (32)
            CH = IH // 4                         # i2h per chunk (8)
            a_t = sb.tile([128, IH, Q, IL], bf16)  # [p, i2h, jq, i2l]
            a_s = sb.tile([128, IH, Q, IL], bf16)  # alpha-scaled
            b_t = sb.tile([128, IH, F, IL], bf16)  # [p, i2h, jf, i2l]

            # DRAM view: row i = 32b + 4s + g, col j = q*64 + f;
            # psum partition = 32g + q, psum free = 64s + f.  With g the LOW
            # row bits, the DRAM dims (g, q) merge to one stride-64 dim, so
            # the whole bank drains in ONE 3-dim dma_start.
            oview = out_ext[:].rearrange(
                "(b s g) (q f) -> b g q s f", b=8, s=8, g=4, q=Q, f=F
            )

            banks = [None] * 8
            rb_ps = []

            def emit_chunk(c):
                ih = slice(CH * c, CH * c + CH)
                nc.vector.tensor_tensor(
                    out=a_t[:, ih],
                    in0=ph_v[:, ih].unsqueeze(2).to_broadcast([128, CH, Q, IL]),
                    in1=iq_t[:].unsqueeze(1).to_broadcast([128, CH, Q, IL]),
                    op=IsEq,
                )
                if c == 0:
                    # alpha chain between the two builds: altp has landed by
                    # the time A0 retires, and e_t is ready before a_s0.
                    nc.vector.reciprocal(out=rt_t[:], in_=tp_t[:])
                    nc.scalar.activation(
                        out=e_t[:], in_=al_t[:], func=Exp, scale=rt_t[:]
                    )
                    nc.tensor.matmul(
                        sum_ps[:], lhsT=e_t[:], rhs=half_col[:],
                        start=True, stop=True,
                    )
                nc.vector.tensor_tensor(
                    out=b_t[:, ih],
                    in0=pl_v[:, ih].unsqueeze(2).to_broadcast([128, CH, F, IL]),
                    in1=if_t[:].unsqueeze(1).to_broadcast([128, CH, F, IL]),
                    op=IsEq,
                )
                as_eng = (
                    nc.gpsimd
                    if os.environ.get("KERNEL_AS_GP", "0") == "1"
                    else nc.vector
                )
                as_eng.tensor_scalar(
                    out=a_s[:, ih], in0=a_t[:, ih], scalar1=e_t[:],
                    scalar2=None, op0=mybir.AluOpType.mult,
                )
                if c == 0:
                    # softmax tail fully BEFORE the row matmuls so rs_t is
                    # ready when bank 0 drains (sum_ps done during builds).
                    nc.vector.reciprocal(out=r_t[:], in_=sum_ps[:])
                    rb = psmax.tile([128, 1], fp32, tag="smax", name="rb_ps")
                    rb_ps.append(rb)
                    nc.tensor.matmul(
                        rb[:], lhsT=ones_row[:], rhs=r_t[:],
                        start=True, stop=True,
                    )
                    nc.vector.tensor_copy(out=rs_t[:], in_=rb[:])
                for h in range(2):
                    banks[c + 4 * h] = pp.tile(
                        [128, 8, F], fp32, tag="bank", name=f"bank{c}_{h}"
                    )
                # h OUTER: consecutive matmuls stay in one row-group, so the
                # PE runs a single serialized stream (g-alternation only
                # overlaps LDW).  Concurrent h-interleaved streams are faster
                # on paper but saturate SBUF read bw and throttle DVE/ACT
                # ops 2-3x (measured), a net loss.
                for h in range(2):
                    for s in range(8):
                        for g in range(4):
                            # row r = 128h + i2, i2 = 4*(CH*c+s) + g
                            kp = slice(64 * h, 64 * h + 64)
                            nc.tensor.matmul(
                                banks[c + 4 * h][32 * g : 32 * g + 32, s],
                                lhsT=a_s[kp, CH * c + s, :, g],
                                rhs=b_t[kp, CH * c + s, :, g],
                                start=True,
                                stop=True,
                                tile_position=(64 * h, 32 * g),
                            )
            def emit_drain(c):
                # Chunk 0's banks drain in s-halves so the output DMA ramps
                # ~1.5us earlier; later banks go whole (fewer issues).
                for h in range(2):
                    bi = c + 4 * h
                    # all drain DMAs issue from the sync ring: its sequencer
                    # is idle during the drain, while the scalar sequencer
                    # runs the evac COPYs back-to-back.
                    eng = nc.sync
                    parts = ((slice(0, 4), slice(4, 8)) if c == 0
                             else (slice(0, 8),))
                    for sp in parts:
                        nsl = sp.stop - sp.start
                        stage = stp.tile(
                            [128, nsl, F], fp32, tag="stage", name="stage"
                        )
                        nc.scalar.activation(
                            out=stage[:], in_=banks[bi][:, sp], func=Copy,
                            scale=rs_t[:],
                        )
                        eng.dma_start(out=oview[bi][:, :, sp], in_=stage[:])

            for c in range(4):
                emit_chunk(c)
                if c >= 1:
                    emit_drain(c - 1)
            emit_drain(3)

    if not nc.is_finalized():
        nc.finalize()
    return nc


def _prep_inputs(alpha_weights, perm_vectors, temperature):
    a = np.asarray(alpha_weights, dtype=np.float32).reshape(K)
    T = np.asarray(temperature, dtype=np.float32).reshape(())
    perm = np.asarray(perm_vectors).astype(np.int64).reshape(K, N)
    ph = (perm >> 6).astype(np.int16)
    pl = (perm & 63).astype(np.int16)
    al_t = np.concatenate([a, a])[:, None].copy()          # [128, 1]
    tp_t = np.full((128, 1), T, dtype=np.float32)
    in_maps = []
    for c in range(NCORES):
        # partition p = k + 64*h, column i2: row r = 128*h + i2 of this core
        phc = ph[:, c * ROWS : (c + 1) * ROWS].reshape(K, 2, 128)
        plc = pl[:, c * ROWS : (c + 1) * ROWS].reshape(K, 2, 128)
        in_maps.append(
            {
                "ph": phc.transpose(1, 0, 2).reshape(128, 128).copy(),
                "pl": plc.transpose(1, 0, 2).reshape(128, 128).copy(),
                "altp": np.concatenate([al_t, tp_t], axis=1).copy(),
            }
        )
    return in_maps


def _install_ntff_hook():
    """Provide antenv.axon_hooks (missing in this image) so that
    run_bass_kernel_spmd(trace=True) can capture NTFF profiles via the
    axon PJRT .so (same mechanism as trn_agent_boot.trn_boot)."""
    import contextlib
    import ctypes
    import types

    try:
        from antenv.axon_hooks import get_axon_ntff_profile_hook  # noqa: F401

        return True
    except ImportError:
        pass
    so_path = "/opt/axon/libaxon_pjrt.so"
    if not os.path.exists(so_path):
        return False
    lib = ctypes.CDLL(so_path)
    if not hasattr(lib, "axon_start_nrt_profile"):
        return False
    lib.axon_start_nrt_profile.argtypes = [
        ctypes.POINTER(ctypes.c_int64),
        ctypes.c_size_t,
    ]
    lib.axon_start_nrt_profile.restype = ctypes.c_int64
    lib.axon_stop_nrt_profile.argtypes = [ctypes.c_char_p]
    lib.axon_stop_nrt_profile.restype = ctypes.c_int64

    @contextlib.contextmanager
    def _hook(output_dir, device_ids):
        import jax

        jax.devices()
        if device_ids:
            ids = (ctypes.c_int64 * len(device_ids))(*device_ids)
            rc = lib.axon_start_nrt_profile(ids, len(device_ids))
        else:
            rc = lib.axon_start_nrt_profile(None, 0)
        if rc != 0:
            raise RuntimeError(f"axon_start_nrt_profile rc={rc}")
        try:
            yield
        finally:
            n = lib.axon_stop_nrt_profile(str(output_dir).encode())
            print(f"ntff profile: {n} file(s) written to {output_dir}")

    import antenv

    mod = types.ModuleType("antenv.axon_hooks")
    mod.get_axon_ntff_profile_hook = lambda: _hook
    mod.set_axon_ntff_profile_hook = lambda h: None
    sys.modules["antenv.axon_hooks"] = mod
    antenv.axon_hooks = mod
    return True


def kernel(alpha_weights, perm_vectors, temperature):
    global LAST_EXEC_NS, LAST_RESULTS
    from concourse.bass_utils import run_bass_kernel_spmd

    if "nc" not in _cached:
        _cached["nc"] = _build_bass()
    nc = _cached["nc"]
    in_maps = _prep_inputs(alpha_weights, perm_vectors, temperature)
    core_ids = list(range(NCORES))
    trace = os.environ.get("KERNEL_TRACE", "0") == "1"
    if trace:
        trace = _install_ntff_hook()
    try:
        res = run_bass_kernel_spmd(nc, in_maps, core_ids, trace=trace)
    except Exception:
        if not trace:
            raise
        res = run_bass_kernel_spmd(nc, in_maps, core_ids, trace=False)
    LAST_EXEC_NS = res.exec_time_ns
    LAST_RESULTS = res
    out = np.concatenate([res.results[c]["out"] for c in range(NCORES)], axis=0)
    return out.astype(np.float32)


if __name__ == "__main__":
    rng = np.random.default_rng(0)
    a = rng.standard_normal(K).astype(np.float32)
    perm = np.stack([rng.permutation(N) for _ in range(K)]).astype(np.int64)
    T = np.ones((), np.float32)
    out = kernel(a, perm, T)
    # numpy reference
    al = np.exp(a / T - (a / T).max())
    al /= al.sum()
    exp = np.zeros((N, N), np.float32)
    np.add.at(exp, (np.broadcast_to(np.arange(N), (K, N)), perm), al[:, None])
    print("max abs err:", np.abs(out - exp).max(), "max ref:", np.abs(exp).max())
    print("exec ns:", LAST_EXEC_NS)


# revision 77
# speedup vs baseline: 2.7879x; 1.0207x over previous
"""AlphaPermutationLayer Trainium2 kernel (v2).

out[i, j] = sum_k softmax(alpha/T)[k] * (perm[k, i] == j),  N=2048, K=64.

Sharding: output ROWS across the 8 cores (row i depends only on perm[:, i]
and alpha — no collective).  Per core (256 rows), digit-split
j = jq*64 + jf (jq in [0,32), jf in [0,64)); one matmul per row i:
    out_i[jq, jf] = sum_k A_i[k, jq] * B_i[k, jf]
with A = alpha-scaled onehot(perm>>6) stationary ([64, 32], LDW 32 cols)
and B = onehot(perm&63) moving ([64, 64]).  Rows are processed two-per-
partition-set: partition p = k + 64*h holds row half h, so DVE one-hot
builds use all 128 lanes while each matmul contracts 64 partitions at
tile_position (64h, 32g) — 4 col-groups give concurrent matmuls.  Single
bf16 pass (no hi/lo): alpha rounding ~2e-3 rel, gate is 2e-2.  PSUM holds
the whole 2MB per-core output; ACT evacuates with the fused 1/S softmax
normalization; strided DMAs (256B runs) write DRAM.
"""

import os
import sys

sys.path.insert(0, "/opt/trn_rl_repo")

import numpy as np

N = 2048
K = 64
NCORES = 8
ROWS = N // NCORES          # 256 rows per core
Q = 32                      # stationary digit width (jq)
F = 64                      # moving digit width (jf)
CW = 32                     # i2 chunk width (4 chunks of 32)
IL = 4                      # low i2 bits kept innermost in one-hot layout

LAST_EXEC_NS = None
LAST_RESULTS = None

_cached = {}


def _build_bass():
    import concourse.tile as tile
    from concourse import bacc, mybir

    fp32 = mybir.dt.float32
    bf16 = mybir.dt.bfloat16
    i16 = mybir.dt.int16
    Copy = mybir.ActivationFunctionType.Copy
    Exp = mybir.ActivationFunctionType.Exp
    IsEq = mybir.AluOpType.is_equal

    nc = bacc.Bacc()

    ph_ext = nc.declare_dram_parameter("ph", [128, 128], i16, isOutput=False)
    pl_ext = nc.declare_dram_parameter("pl", [128, 128], i16, isOutput=False)
    at_ext = nc.declare_dram_parameter("altp", [128, 2], fp32, isOutput=False)
    out_ext = nc.declare_dram_parameter("out", [ROWS, N], fp32, isOutput=True)

    with tile.TileContext(nc) as tc:
        with (
            tc.tile_pool(name="sbuf", bufs=1) as sb,
            tc.tile_pool(name="stage", bufs=10) as stp,
            tc.tile_pool(name="smax_psum", bufs=1, space="PSUM") as psmax,
            tc.tile_pool(name="psum", bufs=7, space="PSUM") as pp,
        ):
            # ---- input loads: 2 DMAs; iotas generated on-chip ---------------
            # One-hot tensors are laid out [p, i2h, digit, i2l] with the LOW
            # i2 bits innermost (IL=4): the is_equal in0 is just ph/pl viewed
            # [p, i2h, 1->digit, i2l] — real data, stride-1 inner, 2x DVE
            # mode with NO host expansion.  The matmul then reads digit cols
            # at stride IL*2 = 8B (2 per 16B SBUF line), cutting the line
            # thrash that throttles concurrent DVE/ACT ops.
            ph_t = sb.tile([128, 128], i16)
            pl_t = sb.tile([128, 128], i16)
            at_t = sb.tile([128, 2], fp32)
            nc.sync.dma_start(out=ph_t[:, 0:64], in_=ph_ext[:, 0:64])
            nc.scalar.dma_start(out=ph_t[:, 64:128], in_=ph_ext[:, 64:128])
            nc.sync.dma_start(out=pl_t[:], in_=pl_ext[:])
            nc.scalar.dma_start(out=at_t[:], in_=at_ext[:])
            ph_v = ph_t[:].rearrange("p (ih il) -> p ih il", il=IL)
            pl_v = pl_t[:].rearrange("p (ih il) -> p ih il", il=IL)
            iq_t = sb.tile([128, Q, IL], i16)   # [p, q, il] = q
            if_t = sb.tile([128, F, IL], i16)   # [p, f, il] = f
            nc.gpsimd.iota(iq_t[:], pattern=[[1, Q], [0, IL]], channel_multiplier=0)
            nc.gpsimd.iota(if_t[:], pattern=[[1, F], [0, IL]], channel_multiplier=0)
            al_t = at_t[:, 0:1]
            tp_t = at_t[:, 1:2]

            # ---- softmax head ----------------------------------------------
            # e = exp(alpha/T) unnormalized; S recovered via matmul with 0.5
            # (partitions hold k twice), 1/S applied at evacuation.
            rt_t = sb.tile([128, 1], fp32)
            e_t = sb.tile([128, 1], fp32)
            ln2_t = sb.tile([128, 1], fp32)
            prime_t = sb.tile([128, 1], fp32)
            half_col = sb.tile([128, 1], fp32)
            ones_row = sb.tile([1, 128], fp32)
            r_t = sb.tile([1, 1], fp32)
            rs_t = sb.tile([128, 1], fp32)
            scr_t = sb.tile([128, 512], bf16)   # warmup scratch (uninit junk)
            nc.vector.memset(scr_t[:, 0:2], 1.0)
            nc.vector.memset(ln2_t[:], float(np.log(2.0)))
            nc.vector.memset(half_col[:], 0.5)
            nc.vector.memset(ones_row[:], 1.0)
            # dep-free ACT op: hoists the one-time activation-table load off
            # the exp critical path.
            nc.scalar.activation(out=prime_t[:], in_=ln2_t[:], func=Exp)
            warm_ps = psmax.tile([1, 512], fp32, tag="smax")
            sum_ps = psmax.tile([1, 1], fp32, tag="smax")
            # HAM pre-warm: a few WIDE dep-free matmuls (~3us of PE busy in 7
            # instructions) so the clock gate reaches 8/8 before the real
            # stream without clogging the PE queue ahead of the softmax sum.
            for _ in range(7):
                nc.tensor.matmul(
                    warm_ps[:], lhsT=scr_t[:, 0:1], rhs=scr_t[:],
                    start=True, stop=True,
                )

            IH = 128 // IL                       # i2h extent (32)
            CH = IH // 4                         # i2h per chunk (8)
            a_t = sb.tile([128, IH, Q, IL], bf16)  # [p, i2h, jq, i2l]
            a_s = sb.tile([128, IH, Q, IL], bf16)  # alpha-scaled
            b_t = sb.tile([128, IH, F, IL], bf16)  # [p, i2h, jf, i2l]

            # DRAM view: row i = 32b + 4s + g, col j = q*64 + f;
            # psum partition = 32g + q, psum free = 64s + f.  With g the LOW
            # row bits, the DRAM dims (g, q) merge to one stride-64 dim, so
            # the whole bank drains in ONE 3-dim dma_start.
            oview = out_ext[:].rearrange(
                "(b s g) (q f) -> b g q s f", b=8, s=8, g=4, q=Q, f=F
            )

            banks = [None] * 8
            rb_ps = []

            def emit_chunk(c):
                ih = slice(CH * c, CH * c + CH)
                nc.vector.tensor_tensor(
                    out=a_t[:, ih],
                    in0=ph_v[:, ih].unsqueeze(2).to_broadcast([128, CH, Q, IL]),
                    in1=iq_t[:].unsqueeze(1).to_broadcast([128, CH, Q, IL]),
                    op=IsEq,
                )
                if c == 0:
                    # alpha chain between the two builds: altp has landed by
                    # the time A0 retires, and e_t is ready before a_s0.
                    nc.vector.reciprocal(out=rt_t[:], in_=tp_t[:])
                    nc.scalar.activation(
                        out=e_t[:], in_=al_t[:], func=Exp, scale=rt_t[:]
                    )
                    nc.tensor.matmul(
                        sum_ps[:], lhsT=e_t[:], rhs=half_col[:],
                        start=True, stop=True,
                    )
                nc.vector.tensor_tensor(
                    out=b_t[:, ih],
                    in0=pl_v[:, ih].unsqueeze(2).to_broadcast([128, CH, F, IL]),
                    in1=if_t[:].unsqueeze(1).to_broadcast([128, CH, F, IL]),
                    op=IsEq,
                )
                as_eng = (
                    nc.gpsimd
                    if os.environ.get("KERNEL_AS_GP", "0") == "1"
                    else nc.vector
                )
                as_eng.tensor_scalar(
                    out=a_s[:, ih], in0=a_t[:, ih], scalar1=e_t[:],
                    scalar2=None, op0=mybir.AluOpType.mult,
                )
                if c == 0:
                    # softmax tail fully BEFORE the row matmuls so rs_t is
                    # ready when bank 0 drains (sum_ps done during builds).
                    nc.vector.reciprocal(out=r_t[:], in_=sum_ps[:])
                    rb = psmax.tile([128, 1], fp32, tag="smax", name="rb_ps")
                    rb_ps.append(rb)
                    nc.tensor.matmul(
                        rb[:], lhsT=ones_row[:], rhs=r_t[:],
                        start=True, stop=True,
                    )
                    nc.vector.tensor_copy(out=rs_t[:], in_=rb[:])
                for h in range(2):
                    banks[c + 4 * h] = pp.tile(
                        [128, 8, F], fp32, tag="bank", name=f"bank{c}_{h}"
                    )
                # h OUTER: consecutive matmuls stay in one row-group, so the
                # PE runs a single serialized stream (g-alternation only
                # overlaps LDW).  Concurrent h-interleaved streams are faster
                # on paper but saturate SBUF read bw and throttle DVE/ACT
                # ops 2-3x (measured), a net loss.
                for h in range(2):
                    for s in range(8):
                        for g in range(4):
                            # row r = 128h + i2, i2 = 4*(CH*c+s) + g
                            kp = slice(64 * h, 64 * h + 64)
                            nc.tensor.matmul(
                                banks[c + 4 * h][32 * g : 32 * g + 32, s],
                                lhsT=a_s[kp, CH * c + s, :, g],
                                rhs=b_t[kp, CH * c + s, :, g],
                                start=True,
                                stop=True,
                                tile_position=(64 * h, 32 * g),
                            )
            def emit_drain(c):
                # Chunk 0's banks drain in s-halves so the output DMA ramps
                # ~1.5us earlier; later banks go whole (fewer issues).
                for h in range(2):
                    bi = c + 4 * h
                    # all drain DMAs issue from the sync ring: its sequencer
                    # is idle during the drain, while the scalar sequencer
                    # runs the evac COPYs back-to-back.
                    eng = nc.sync
                    parts = ((slice(0, 4), slice(4, 8)) if c in (0, 3)
                             else (slice(0, 8),))
                    for sp in parts:
                        nsl = sp.stop - sp.start
                        stage = stp.tile(
                            [128, nsl, F], fp32, tag="stage", name="stage"
                        )
                        nc.scalar.activation(
                            out=stage[:], in_=banks[bi][:, sp], func=Copy,
                            scale=rs_t[:],
                        )
                        eng.dma_start(out=oview[bi][:, :, sp], in_=stage[:])

            for c in range(4):
                emit_chunk(c)
                if c >= 1:
                    emit_drain(c - 1)
            emit_drain(3)

    if not nc.is_finalized():
        nc.finalize()
    return nc


def _prep_inputs(alpha_weights, perm_vectors, temperature):
    a = np.asarray(alpha_weights, dtype=np.float32).reshape(K)
    T = np.asarray(temperature, dtype=np.float32).reshape(())
    perm = np.asarray(perm_vectors).astype(np.int64).reshape(K, N)
    ph = (perm >> 6).astype(np.int16)
    pl = (perm & 63).astype(np.int16)
    al_t = np.concatenate([a, a])[:, None].copy()          # [128, 1]
    tp_t = np.full((128, 1), T, dtype=np.float32)
    in_maps = []
    for c in range(NCORES):
        # partition p = k + 64*h, column i2: row r = 128*h + i2 of this core
        phc = ph[:, c * ROWS : (c + 1) * ROWS].reshape(K, 2, 128)
        plc = pl[:, c * ROWS : (c + 1) * ROWS].reshape(K, 2, 128)
        in_maps.append(
            {
                "ph": phc.transpose(1, 0, 2).reshape(128, 128).copy(),
                "pl": plc.transpose(1, 0, 2).reshape(128, 128).copy(),
                "altp": np.concatenate([al_t, tp_t], axis=1).copy(),
            }
        )
    return in_maps


def _install_ntff_hook():
    """Provide antenv.axon_hooks (missing in this image) so that
    run_bass_kernel_spmd(trace=True) can capture NTFF profiles via the
    axon PJRT .so (same mechanism as trn_agent_boot.trn_boot)."""
    import contextlib
    import ctypes
    import types

    try:
        from antenv.axon_hooks import get_axon_ntff_profile_hook  # noqa: F401

        return True
    except ImportError:
        pass
    so_path = "/opt/axon/libaxon_pjrt.so"
    if not os.path.exists(so_path):
        return False
    lib = ctypes.CDLL(so_path)
    if not hasattr(lib, "axon_start_nrt_profile"):
        return False
    lib.axon_start_nrt_profile.argtypes = [
        ctypes.POINTER(ctypes.c_int64),
        ctypes.c_size_t,
    ]
    lib.axon_start_nrt_profile.restype = ctypes.c_int64
    lib.axon_stop_nrt_profile.argtypes = [ctypes.c_char_p]
    lib.axon_stop_nrt_profile.restype = ctypes.c_int64

    @contextlib.contextmanager
    def _hook(output_dir, device_ids):
        import jax

        jax.devices()
        if device_ids:
            ids = (ctypes.c_int64 * len(device_ids))(*device_ids)
            rc = lib.axon_start_nrt_profile(ids, len(device_ids))
        else:
            rc = lib.axon_start_nrt_profile(None, 0)
        if rc != 0:
            raise RuntimeError(f"axon_start_nrt_profile rc={rc}")
        try:
            yield
        finally:
            n = lib.axon_stop_nrt_profile(str(output_dir).encode())
            print(f"ntff profile: {n} file(s) written to {output_dir}")

    import antenv

    mod = types.ModuleType("antenv.axon_hooks")
    mod.get_axon_ntff_profile_hook = lambda: _hook
    mod.set_axon_ntff_profile_hook = lambda h: None
    sys.modules["antenv.axon_hooks"] = mod
    antenv.axon_hooks = mod
    return True


def kernel(alpha_weights, perm_vectors, temperature):
    global LAST_EXEC_NS, LAST_RESULTS
    from concourse.bass_utils import run_bass_kernel_spmd

    if "nc" not in _cached:
        _cached["nc"] = _build_bass()
    nc = _cached["nc"]
    in_maps = _prep_inputs(alpha_weights, perm_vectors, temperature)
    core_ids = list(range(NCORES))
    trace = os.environ.get("KERNEL_TRACE", "0") == "1"
    if trace:
        trace = _install_ntff_hook()
    try:
        res = run_bass_kernel_spmd(nc, in_maps, core_ids, trace=trace)
    except Exception:
        if not trace:
            raise
        res = run_bass_kernel_spmd(nc, in_maps, core_ids, trace=False)
    LAST_EXEC_NS = res.exec_time_ns
    LAST_RESULTS = res
    out = np.concatenate([res.results[c]["out"] for c in range(NCORES)], axis=0)
    return out.astype(np.float32)


if __name__ == "__main__":
    rng = np.random.default_rng(0)
    a = rng.standard_normal(K).astype(np.float32)
    perm = np.stack([rng.permutation(N) for _ in range(K)]).astype(np.int64)
    T = np.ones((), np.float32)
    out = kernel(a, perm, T)
    # numpy reference
    al = np.exp(a / T - (a / T).max())
    al /= al.sum()
    exp = np.zeros((N, N), np.float32)
    np.add.at(exp, (np.broadcast_to(np.arange(N), (K, N)), perm), al[:, None])
    print("max abs err:", np.abs(out - exp).max(), "max ref:", np.abs(exp).max())
    print("exec ns:", LAST_EXEC_NS)
